# revision 1
# baseline (speedup 1.0000x reference)
"""Trainium2 Bass kernel for the DeformationGraph problem.

Math: the reference computes, per batch b and vertex v,
    out[b,v,k] = sum_c W[v,c] * ( sum_d (X[b,v,d]-center[b,c,d]) * R[b,c,k,d]
                                  + center[b,c,k] + V_nodes[b,c,k] )
which factors into a vertex-independent per-node affine map:
    t[b,c,k]   = center[b,c,k] + V_nodes[b,c,k] - sum_d center[b,c,d]*R[b,c,k,d]
    out[b,v,k] = sum_d X[b,v,d] * (W @ R[..,k,d])[v]  +  (W @ t[..,k])[v]
i.e. one (V,C)@(C,48) matmul Y = W @ G, then a per-vertex contraction of Y
with [X,1].  The big tensors (W: 32MB, X, out) are sharded over the vertex
dimension across the 8 cores; G is replicated.

Layout: the 48 live Y rows sit at partitions j = d*16 + (k*4 + b), d in
0..3 (d==3 = translation/ones slot), rows 12..15 of each 16-block zero.
The 16-stride makes both halves of the d-reduction 32-aligned, which the
engines need, while keeping xd a single DMA.

fp32 matmul on TRN2 runs in LOW_HIGH dual-pass mode (~5x slower), so the
matmul uses the exact-enough 3-term bf16 split:
    W @ G ~= Wh@Gh + Wl@Gh + Wh@Gl     (Wh=bf16(W), Wl=bf16(W-Wh), ...)
measured end-to-end error vs the fp32 reference: ~4e-6 absmax.

The contraction dim C=160 splits into an A part (c 0..127, K=128) and a B
part (c 128..159, K=32).  The three B-part terms are packed into one K=96
matmul by stacking [WhB; WhB; WlB] against [GhB; GlB; GhB] host-side.

Reduction: engine 2-input ops need equal base partitions for SBUF+SBUF
pairs but allow arbitrary bases for mixed PSUM+SBUF pairs, so per sub-chunk:
    DVE   p PSUM = y * xd
    ACT   q (32,n) SBUF  = copy p[32:64]
    DVE   a32 (32,n) SBUF = p[0:32] + q          (d0+d2 | d1+d3)
and the last level runs on the DMA engines (CCE add at the DRAM dest):
    DMA   outT[:, m]  = a32[0:12]   (HWDGE store)
    DMA   outT[:, m] += a32[16:28]  (SWDGE accumulate, dep-chained)

DMA macro chunks ramp up so compute starts early, then amortize the
~0.7us per-DMA sequencer issue cost; compute runs in 512-wide sub-chunks
(PSUM budget); a ~3.5us dummy-matmul warmup runs during the first DMAs to
lift the PE out of its cold 1.2GHz HAM state.
"""

import numpy as np
import ml_dtypes

import concourse.mybir as mybir
import concourse.tile as tile
from concourse import bacc
from concourse.bass_utils import run_bass_kernel_spmd
from concourse.tile_rust import add_dep_helper

B, V, C = 4, 50000, 160
N_CORES = 8
VS = V // N_CORES            # 6250 vertices per core
VSP = 6272                   # padded vertex shard
MACROS = [512, 1024, 2048, 2048, 512, 128]
SUB = 512
F32 = mybir.dt.float32
BF16 = mybir.dt.bfloat16
NPBF16 = ml_dtypes.bfloat16


def _build_bass():
    nc = bacc.Bacc()

    wha_d = nc.dram_tensor("wha", [128, VSP], BF16, kind="ExternalInput")
    wla_d = nc.dram_tensor("wla", [128, VSP], BF16, kind="ExternalInput")
    wb_d = nc.dram_tensor("wb", [96, VSP], BF16, kind="ExternalInput")
    xd_d = nc.dram_tensor("xd", [64, VSP], F32, kind="ExternalInput")
    gh0_d = nc.dram_tensor("gh0", [128, 64], BF16, kind="ExternalInput")
    gl0_d = nc.dram_tensor("gl0", [128, 64], BF16, kind="ExternalInput")
    gbk_d = nc.dram_tensor("gbk", [96, 64], BF16, kind="ExternalInput")
    outT = nc.dram_tensor("outT", [12, VSP], F32, kind="ExternalOutput")

    with tile.TileContext(nc) as tc:
        with (
            tc.tile_pool(name="gpool", bufs=1) as gpool,
            tc.tile_pool(name="wpool", bufs=5) as wpool,
            tc.tile_pool(name="xpool", bufs=2) as xpool,
            tc.tile_pool(name="qpool", bufs=3) as qpool,
            tc.tile_pool(name="apool", bufs=2) as apool,
            tc.tile_pool(name="ypool", bufs=4, space="PSUM") as ypool,
            tc.tile_pool(name="ppool", bufs=2, space="PSUM") as ppool,
        ):
            gh0 = gpool.tile([128, 64], BF16)
            nc.sync.dma_start(out=gh0[:], in_=gh0_d[:])
            gl0 = gpool.tile([128, 64], BF16)
            nc.sync.dma_start(out=gl0[:], in_=gl0_d[:])
            gbk = gpool.tile([96, 64], BF16)
            nc.sync.dma_start(out=gbk[:], in_=gbk_d[:])

            # PE HAM warmup (output never read)
            wsc = gpool.tile([128, 512], BF16)
            nc.vector.memset(wsc[:], 0.0)
            ywarm = ypool.tile([64, 512], F32, tag="ywarm", bufs=1)
            for w in range(12):
                nc.tensor.matmul(ywarm[:, :], gh0[:], wsc[:, :],
                                 start=(w == 0), stop=(w == 11),
                                 skip_group_check=True)

            a32 = apool.tile([32, VSP], F32, bufs=1)

            m0 = 0
            for mn in MACROS:
                msl = slice(m0, m0 + mn)
                wha = wpool.tile([128, mn], BF16, tag="wha")
                nc.sync.dma_start(out=wha[:], in_=wha_d[:, msl])
                wla = wpool.tile([128, mn], BF16, tag="wla")
                nc.sync.dma_start(out=wla[:], in_=wla_d[:, msl])
                bpk = wpool.tile([96, mn], BF16, tag="bpk")
                nc.sync.dma_start(out=bpk[:], in_=wb_d[:, msl])
                xdt = xpool.tile([64, mn], F32, tag="xdt", bufs=3)
                nc.gpsimd.dma_start(out=xdt[:], in_=xd_d[:, msl])

                # process sub-chunks in PAIRS: the even sub-chunk's matmul
                # group runs in PE column-group 0 (PSUM partitions 0:64),
                # the odd one's in column-group 64 — interleaved issue makes
                # the two groups stream concurrently through the array
                # (M=64 uses only half the PE columns otherwise).
                for u0 in range(0, mn, 2 * SUB):
                    n1 = min(SUB, mn - u0)
                    n2 = min(SUB, mn - u0 - n1)
                    u1 = u0 + n1
                    y = ypool.tile([128, SUB], F32, tag="y")
                    terms = ((gh0, wha), (gh0, wla), (gl0, wha), (gbk, bpk))
                    for t, (g, w) in enumerate(terms):
                        nc.tensor.matmul(y[0:64, 0:n1], g[:],
                                         w[:, u0:u0 + n1],
                                         start=(t == 0), stop=(t == 3),
                                         skip_group_check=True)
                        if n2:
                            nc.tensor.matmul(y[64:128, 0:n2], g[:],
                                             w[:, u1:u1 + n2],
                                             start=(t == 0), stop=(t == 3),
                                             skip_group_check=True)

                    p = ppool.tile([128, SUB], F32, tag="p")
                    nc.vector.tensor_mul(out=p[0:64, 0:n1], in0=y[0:64, 0:n1],
                                         in1=xdt[:, u0:u0 + n1])
                    q = qpool.tile([32, n1], F32, tag="q")
                    nc.scalar.copy(out=q[:], in_=p[32:64, 0:n1])
                    nc.vector.tensor_add(out=a32[:, m0 + u0:m0 + u0 + n1],
                                         in0=p[0:32, 0:n1], in1=q[:])
                    if n2:
                        nc.vector.tensor_mul(out=p[64:128, 0:n2],
                                             in0=y[64:128, 0:n2],
                                             in1=xdt[:, u1:u1 + n2])
                        q2 = qpool.tile([32, n2], F32, tag="q2")
                        nc.scalar.copy(out=q2[:], in_=p[96:128, 0:n2])
                        nc.vector.tensor_add(out=a32[:, m0 + u1:m0 + u1 + n2],
                                             in0=p[64:96, 0:n2], in1=q2[:])

                m0 += mn

            m0 = 0
            for mn in MACROS:
                msl = slice(m0, m0 + mn)
                d0 = nc.sync.dma_start(out=outT[:, msl], in_=a32[0:12, msl])
                d1 = nc.gpsimd.dma_start(out=outT[:, msl],
                                         in_=a32[16:28, msl],
                                         accum_op=mybir.AluOpType.add)
                add_dep_helper(d1.ins, d0.ins,
                               reason="serialize DRAM accumulate after store")
                m0 += mn
    nc.finalize()
    return nc


_NC_CACHE = None


def _get_nc():
    global _NC_CACHE
    if _NC_CACHE is None:
        _NC_CACHE = _build_bass()
    return _NC_CACHE


def _host_prep(X, V_nodes, rot6d_nodes, W_nodes, idx_nn_to_nodes):
    """Small per-node math (B*C=640 rows) + shard/layout of the big tensors."""
    X = np.asarray(X, np.float32)
    Vn = np.asarray(V_nodes, np.float32)
    d6 = np.asarray(rot6d_nodes, np.float32)
    W = np.asarray(W_nodes, np.float32)
    idx = np.asarray(idx_nn_to_nodes).astype(np.int64)

    a1, a2 = d6[..., :3], d6[..., 3:]
    eps = np.float32(1e-8)
    n1 = np.sqrt(np.sum(a1 * a1, -1, keepdims=True, dtype=np.float32))
    b1 = a1 / np.maximum(n1, eps)
    dot = np.sum(b1 * a2, -1, keepdims=True, dtype=np.float32)
    a2p = a2 - dot * b1
    n2 = np.sqrt(np.sum(a2p * a2p, -1, keepdims=True, dtype=np.float32))
    b2 = a2p / np.maximum(n2, eps)
    b3 = np.cross(b1, b2)
    R = np.stack([b1, b2, b3], axis=-2).astype(np.float32)  # (B,C,3,3) [b,c,k,d]

    center = X[:, idx, :]                                   # (B,C,3)
    t = (center + Vn - np.einsum('bcd,bckd->bck', center, R)).astype(np.float32)

    # G columns at j = d*16 + k*4 + b; cols 12..15 of each block zero
    G = np.zeros((C, 64), np.float32)
    for d in range(4):
        for k in range(3):
            for b in range(B):
                j = d * 16 + k * 4 + b
                G[:, j] = R[b, :, k, d] if d < 3 else t[b, :, k]

    Gh = G.astype(NPBF16)
    Gl = (G - Gh.astype(np.float32)).astype(NPBF16)
    gh0 = np.ascontiguousarray(Gh[0:128])
    gl0 = np.ascontiguousarray(Gl[0:128])
    gbk = np.ascontiguousarray(
        np.concatenate([Gh[128:160], Gl[128:160], Gh[128:160]], axis=0))

    Wh = W.astype(NPBF16)
    Wl = (W - Wh.astype(np.float32)).astype(NPBF16)

    in_maps = []
    for i in range(N_CORES):
        vsl = slice(i * VS, (i + 1) * VS)
        wht = np.zeros((160, VSP), NPBF16)
        wht[:, :VS] = Wh[vsl].T
        wlt = np.zeros((160, VSP), NPBF16)
        wlt[:, :VS] = Wl[vsl].T
        wha = np.ascontiguousarray(wht[0:128])
        wla = np.ascontiguousarray(wlt[0:128])
        wb = np.ascontiguousarray(
            np.concatenate([wht[128:160], wht[128:160], wlt[128:160]], axis=0))
        # xd rows d*16 + k*4 + b: X[b,:,d] for d<3, ones for d==3
        xd = np.zeros((64, VSP), np.float32)
        for d in range(4):
            for k in range(3):
                for b in range(B):
                    r = d * 16 + k * 4 + b
                    xd[r, :VS] = X[b, vsl, d] if d < 3 else 1.0
        in_maps.append({"wha": wha, "wla": wla, "wb": wb, "xd": xd,
                        "gh0": gh0, "gl0": gl0, "gbk": gbk})
    return in_maps


def _gather(results):
    out = np.empty((B, V, 3), np.float32)
    for i, res in enumerate(results):
        oT = res["outT"]
        vsl = slice(i * VS, (i + 1) * VS)
        for k in range(3):
            for b in range(4):
                out[b, vsl, k] = oT[k * 4 + b, :VS]
    return out


def kernel(X, V_nodes, rot6d_nodes, W_nodes, idx_nn_to_nodes, **run_kwargs):
    in_maps = _host_prep(X, V_nodes, rot6d_nodes, W_nodes, idx_nn_to_nodes)
    res = run_bass_kernel_spmd(_get_nc(), in_maps,
                               core_ids=list(range(N_CORES)), **run_kwargs)
    out = _gather(res.results)
    kernel.last_run = res
    return out



# revision 6
# speedup vs baseline: 1.2192x; 1.2192x over previous
"""Trainium2 Bass kernel for the DeformationGraph problem.

Math: per batch b and vertex v,
    out[b,v,k] = sum_c W[v,c] * ( sum_d (X[b,v,d]-center[b,c,d]) * R[b,c,k,d]
                                  + center[b,c,k] + V_nodes[b,c,k] )
factors into a vertex-independent per-node affine map:
    t[b,c,k]   = center[b,c,k] + V_nodes[b,c,k] - sum_d center[b,c,d]*R[b,c,k,d]
    out[b,v,k] = sum_d X[b,v,d] * (W @ R[..,k,d])[v]  +  (W @ t[..,k])[v]
i.e. one (V,C)@(C,48) matmul Y = W @ G, then a per-vertex contraction of Y
with [X,1].  W/X/out are sharded over the vertex dim across 8 cores.

Precision: rel-err budget is 2e-2; a single bf16 term (W, G, x, and the
product tensor all bf16, fp32 accumulation) measures ~3e-3 end-to-end,
so no multi-term splits are used.

Per-core pipeline (vertex shard padded to 6272 = 6*1024 + 128):
  - PE: per 1024-vertex pair, y[0:64] and y[64:128] in one PSUM tile get
    (K=128 "A" + K=32 "B") accumulated matmuls (G-column layout
    j = k*16 + d*4 + b, d==3 = translation, cols 48:64 zero).
  - DVE: one [128,512] tensor_mul  s = y * xd2  (bf16 out to SBUF).
    xd2 is the compact per-vertex [X,1] table replicated 4x along
    partitions by SBUF->SBUF DMAs so the multiply is partition-tall
    (engine op cost scales with free-dim columns only).
  - PE again: the 4-way d-reduction runs as a 0/1 "reduction matmul"
    r[24, n] = RED^T @ s  (rows h*12 + k*4 + b), output PSUM, DMA'd
    straight to DRAM.  This keeps DVE at one op per 1024 vertices and
    leaves ACT/Pool free for DMA issue.
HBM traffic/core: ~1.6MB W_A + 0.4MB W_B + 0.2MB x + 0.3MB out = 2.5MB.
"""

import numpy as np
import ml_dtypes

import concourse.mybir as mybir
import concourse.tile as tile
from concourse import bacc
from concourse.bass_utils import run_bass_kernel_spmd

B, V, C = 4, 50000, 160
N_CORES = 8
VS = V // N_CORES            # 6250 vertices per core
VSP = 6272                   # padded shard: 6 pairs of 1024 + 128 tail
NPAIR = 6
PC = 3200                    # pair-col space: 6*512 + 128
F32 = mybir.dt.float32
BF16 = mybir.dt.bfloat16
NPBF16 = ml_dtypes.bfloat16

WCH = [(0, 2048), (2048, 4096), (4096, VSP)]   # wha DMA chunks
BCH = [(0, 3072), (3072, VSP)]                 # whb DMA chunks
N_WARM = 8


def _locate(tiles, chunks, g0, width):
    for t, (c0, c1) in zip(tiles, chunks):
        if c0 <= g0 and g0 + width <= c1:
            return t, slice(g0 - c0, g0 - c0 + width)
    raise AssertionError(f"col range {g0}+{width} crosses chunk boundary")


def _build_bass():
    nc = bacc.Bacc()

    cst_d = nc.dram_tensor("cst", [128, 160], BF16, kind="ExternalInput")
    wha_d = nc.dram_tensor("wha", [128, VSP], BF16, kind="ExternalInput")
    whb_d = nc.dram_tensor("whb", [32, VSP], BF16, kind="ExternalInput")
    xc_d = nc.dram_tensor("xc", [32, PC], BF16, kind="ExternalInput")
    outT = nc.dram_tensor("outT", [24, PC], F32, kind="ExternalOutput")

    with tile.TileContext(nc) as tc:
        with (
            tc.tile_pool(name="cpool", bufs=1) as cpool,
            tc.tile_pool(name="spool", bufs=3) as spool,
            tc.tile_pool(name="ypool", bufs=2, space="PSUM") as ypool,
            tc.tile_pool(name="rpool", bufs=2, space="PSUM") as rpool,
        ):
            cst = cpool.tile([128, 160], BF16)
            nc.sync.dma_start(out=cst[:], in_=cst_d[:])
            xct = cpool.tile([32, PC], BF16)
            nc.sync.dma_start(out=xct[:], in_=xc_d[:])

            ghA = cst[:, 0:64]
            RED24 = cst[:, 64:88]
            ghB = cst[0:32, 96:160]

            wha_t = []
            for i, (c0, c1) in enumerate(WCH):
                t = cpool.tile([128, c1 - c0], BF16, tag=f"wha{i}")
                nc.sync.dma_start(out=t[:], in_=wha_d[:, c0:c1])
                wha_t.append(t)
            whb_t = []
            for i, (c0, c1) in enumerate(BCH):
                t = cpool.tile([32, c1 - c0], BF16, tag=f"whb{i}")
                nc.gpsimd.dma_start(out=t[:], in_=whb_d[:, c0:c1])
                whb_t.append(t)

            # xd2: compact x table replicated 4x along partitions so the
            # multiply runs partition-tall.  Rows h*64 + k*16 + (d*4+b).
            xd2 = cpool.tile([128, PC], BF16, tag="xd2")
            for h in range(2):
                for k in range(4):
                    r0 = h * 64 + 16 * k
                    nc.gpsimd.dma_start(out=xd2[r0:r0 + 16, :],
                                        in_=xct[16 * h:16 * h + 16, :])

            # PE HAM warmup (output never read)
            wsc = cpool.tile([128, 512], BF16, tag="wsc")
            nc.vector.memset(wsc[:], 0.0)
            ywarm = ypool.tile([64, 512], F32, tag="ywarm", bufs=1)
            for w in range(N_WARM):
                nc.tensor.matmul(ywarm[:], ghA, wsc[:],
                                 start=(w == 0), stop=(w == N_WARM - 1),
                                 skip_group_check=True)

            # DMA cannot read PSUM: RED-matmul results bounce through an
            # SBUF staging buffer (per-pair copies alternate ACT/Pool).
            ro = cpool.tile([24, PC], F32, tag="ro")

            for p in range(NPAIR):
                y = ypool.tile([128, 512], F32, tag="y")
                for h in range(2):
                    g0 = 1024 * p + 512 * h
                    wa, sa = _locate(wha_t, WCH, g0, 512)
                    wb, sb = _locate(whb_t, BCH, g0, 512)
                    nc.tensor.matmul(y[64 * h:64 * h + 64, :], ghA,
                                     wa[:, sa], start=True, stop=False,
                                     skip_group_check=True)
                    nc.tensor.matmul(y[64 * h:64 * h + 64, :], ghB,
                                     wb[:, sb], start=False, stop=True,
                                     skip_group_check=True)
                s = spool.tile([128, 512], BF16, tag="s")
                nc.vector.tensor_mul(out=s[:], in0=y[:],
                                     in1=xd2[:, 512 * p:512 * p + 512])
                r = rpool.tile([24, 512], F32, tag="r", bufs=3)
                nc.tensor.matmul(r[:], RED24, s[:], start=True, stop=True,
                                 skip_group_check=True)
                csl = slice(512 * p, 512 * p + 512)
                nc.scalar.copy(out=ro[:, csl], in_=r[:])
                if p == 2:
                    nc.sync.dma_start(out=outT[:, 0:1536], in_=ro[:, 0:1536])

            # 128-vertex tail (single half)
            yt = ypool.tile([64, 128], F32, tag="yt", bufs=1)
            wa, sa = _locate(wha_t, WCH, 6144, 128)
            wb, sb = _locate(whb_t, BCH, 6144, 128)
            nc.tensor.matmul(yt[:], ghA, wa[:, sa], start=True, stop=False,
                             skip_group_check=True)
            nc.tensor.matmul(yt[:], ghB, wb[:, sb], start=False, stop=True,
                             skip_group_check=True)
            st = spool.tile([64, 128], BF16, tag="st")
            nc.vector.tensor_mul(out=st[:], in0=yt[:],
                                 in1=xd2[0:64, 3072:3200])
            rt = rpool.tile([24, 128], F32, tag="rt", bufs=1)
            nc.tensor.matmul(rt[:], cst[0:64, 64:88], st[:],
                             start=True, stop=True, skip_group_check=True)
            nc.scalar.copy(out=ro[:, 3072:3200], in_=rt[:])
            nc.sync.dma_start(out=outT[:, 1536:3200], in_=ro[:, 1536:3200])
    nc.finalize()
    return nc


_NC_CACHE = None


def _get_nc():
    global _NC_CACHE
    if _NC_CACHE is None:
        _NC_CACHE = _build_bass()
    return _NC_CACHE


def _host_prep(X, V_nodes, rot6d_nodes, W_nodes, idx_nn_to_nodes):
    """Small per-node math (B*C=640 rows) + shard/layout of the big tensors."""
    X = np.asarray(X, np.float32)
    Vn = np.asarray(V_nodes, np.float32)
    d6 = np.asarray(rot6d_nodes, np.float32)
    W = np.asarray(W_nodes, np.float32)
    idx = np.asarray(idx_nn_to_nodes).astype(np.int64)

    a1, a2 = d6[..., :3], d6[..., 3:]
    eps = np.float32(1e-8)
    n1 = np.sqrt(np.sum(a1 * a1, -1, keepdims=True, dtype=np.float32))
    b1 = a1 / np.maximum(n1, eps)
    dot = np.sum(b1 * a2, -1, keepdims=True, dtype=np.float32)
    a2p = a2 - dot * b1
    n2 = np.sqrt(np.sum(a2p * a2p, -1, keepdims=True, dtype=np.float32))
    b2 = a2p / np.maximum(n2, eps)
    b3 = np.cross(b1, b2)
    R = np.stack([b1, b2, b3], axis=-2).astype(np.float32)  # (B,C,3,3) [b,c,k,d]

    center = X[:, idx, :]                                   # (B,C,3)
    t = (center + Vn - np.einsum('bcd,bckd->bck', center, R)).astype(np.float32)

    # G columns at j = k*16 + d*4 + b (d==3 = translation); cols 48:64 zero
    Gv = np.zeros((C, 4, 4, 4), np.float32)
    Gv[:, 0:3, 0:3, :] = np.transpose(R, (1, 2, 3, 0))
    Gv[:, 0:3, 3, :] = np.transpose(t, (1, 2, 0))
    G = Gv.reshape(C, 64)

    RED = np.zeros((2, 4, 4, 4, 24), np.float32)
    for h in range(2):
        for k in range(3):
            for b in range(B):
                RED[h, k, :, b, h * 12 + k * 4 + b] = 1.0
    RED = RED.reshape(128, 24)

    cst = np.zeros((128, 160), NPBF16)
    cst[:, 0:64] = G[0:128].astype(NPBF16)
    cst[:, 64:88] = RED.astype(NPBF16)
    cst[0:32, 96:160] = G[128:160].astype(NPBF16)

    Wb = W.astype(NPBF16)
    in_maps = []
    for i in range(N_CORES):
        vsl = slice(i * VS, (i + 1) * VS)
        wt = np.zeros((160, VSP), NPBF16)
        wt[:, :VS] = Wb[vsl].T
        wha = np.ascontiguousarray(wt[0:128])
        whb = np.ascontiguousarray(wt[128:160])

        Xs = np.zeros((B, VSP, 3), np.float32)
        Xs[:, :VS] = X[:, vsl, :]
        xc = np.zeros((2, 4, 4, PC), np.float32)        # [h, d, b, col]
        main = Xs[:, :6144].reshape(B, NPAIR, 2, 512, 3)
        xc[:, 0:3, :, 0:3072] = np.transpose(
            main, (2, 4, 0, 1, 3)).reshape(2, 3, B, 3072)
        xc[:, 3, :, 0:3072] = 1.0
        xc[0, 0:3, :, 3072:3200] = np.transpose(Xs[:, 6144:6272], (2, 0, 1))
        xc[0, 3, :, 3072:3200] = 1.0
        xc = np.ascontiguousarray(xc.reshape(32, PC).astype(NPBF16))

        in_maps.append({"cst": cst, "wha": wha, "whb": whb, "xc": xc})
    return in_maps


def _gather(results):
    out = np.empty((B, V, 3), np.float32)
    for i, res in enumerate(results):
        o = res["outT"].reshape(2, 3, 4, PC)            # [h, k, b, col]
        om = o[:, :, :, 0:3072].reshape(2, 3, 4, NPAIR, 512)
        block = np.empty((B, VSP, 3), np.float32)
        block[:, :6144] = np.transpose(om, (2, 3, 0, 4, 1)).reshape(B, 6144, 3)
        block[:, 6144:6272] = np.transpose(o[0, :, :, 3072:3200], (1, 2, 0))
        out[:, i * VS:(i + 1) * VS] = block[:, :VS]
    return out


def kernel(X, V_nodes, rot6d_nodes, W_nodes, idx_nn_to_nodes, **run_kwargs):
    in_maps = _host_prep(X, V_nodes, rot6d_nodes, W_nodes, idx_nn_to_nodes)
    res = run_bass_kernel_spmd(_get_nc(), in_maps,
                               core_ids=list(range(N_CORES)), **run_kwargs)
    out = _gather(res.results)
    kernel.last_run = res
    return out


# revision 9
# speedup vs baseline: 1.2535x; 1.0282x over previous
"""Trainium2 Bass kernel for the DeformationGraph problem.

Math: per batch b and vertex v,
    out[b,v,k] = sum_c W[v,c] * ( sum_d (X[b,v,d]-center[b,c,d]) * R[b,c,k,d]
                                  + center[b,c,k] + V_nodes[b,c,k] )
factors into a vertex-independent per-node affine map:
    t[b,c,k]   = center[b,c,k] + V_nodes[b,c,k] - sum_d center[b,c,d]*R[b,c,k,d]
    out[b,v,k] = sum_d X[b,v,d] * (W @ R[..,k,d])[v]  +  (W @ t[..,k])[v]
i.e. one (V,C)@(C,48) matmul Y = W @ G, then a per-vertex contraction of Y
with [X,1].  W/X/out are sharded over the vertex dim across 8 cores.

Precision: rel-err budget is 2e-2; a single bf16 term (W, G, x, and the
product tensor all bf16, fp32 accumulation) measures ~3e-3 end-to-end,
so no multi-term splits are used.

Per-core pipeline (vertex shard padded to 6272 = 6*1024 + 128):
  - PE: per 1024-vertex pair, y[0:64] and y[64:128] in one PSUM tile get
    (K=128 "A" + K=32 "B") accumulated matmuls (G-column layout
    j = k*16 + d*4 + b, d==3 = translation, cols 48:64 zero).
  - DVE: one [128,512] tensor_mul  s = y * xd2  (bf16 out to SBUF).
    xd2 is the compact per-vertex [X,1] table replicated 4x along
    partitions by SBUF->SBUF DMAs so the multiply is partition-tall
    (engine op cost scales with free-dim columns only).
  - PE again: the 4-way d-reduction runs as a 0/1 "reduction matmul"
    r[24, n] = RED^T @ s  (rows h*12 + k*4 + b), output PSUM, DMA'd
    straight to DRAM.  This keeps DVE at one op per 1024 vertices and
    leaves ACT/Pool free for DMA issue.
HBM traffic/core: ~1.6MB W_A + 0.4MB W_B + 0.2MB x + 0.3MB out = 2.5MB.
"""

import numpy as np
import ml_dtypes

import concourse.mybir as mybir
import concourse.tile as tile
from concourse import bacc
from concourse.bass_utils import run_bass_kernel_spmd

B, V, C = 4, 50000, 160
N_CORES = 8
VS = V // N_CORES            # 6250 vertices per core
VSP = 6272                   # padded shard: 6 pairs of 1024 + 128 tail
NPAIR = 6
PC = 3200                    # pair-col space: 6*512 + 128
F32 = mybir.dt.float32
BF16 = mybir.dt.bfloat16
NPBF16 = ml_dtypes.bfloat16

WCH = [(0, 1024), (1024, 3072), (3072, VSP)]   # wha DMA chunks
BCH = [(0, 1024), (1024, VSP)]                 # whb DMA chunks
N_WARM = 24                                    # N=128 ramp matmuls


def _locate(tiles, chunks, g0, width):
    for t, (c0, c1) in zip(tiles, chunks):
        if c0 <= g0 and g0 + width <= c1:
            return t, slice(g0 - c0, g0 - c0 + width)
    raise AssertionError(f"col range {g0}+{width} crosses chunk boundary")


def _build_bass():
    nc = bacc.Bacc()

    cst_d = nc.dram_tensor("cst", [128, 160], BF16, kind="ExternalInput")
    wha_d = nc.dram_tensor("wha", [128, VSP], BF16, kind="ExternalInput")
    whb_d = nc.dram_tensor("whb", [32, VSP], BF16, kind="ExternalInput")
    xc_d = nc.dram_tensor("xc", [32, PC], BF16, kind="ExternalInput")
    outT = nc.dram_tensor("outT", [24, PC], F32, kind="ExternalOutput")

    with tile.TileContext(nc) as tc:
        with (
            tc.tile_pool(name="cpool", bufs=1) as cpool,
            tc.tile_pool(name="spool", bufs=3) as spool,
            tc.tile_pool(name="ypool", bufs=2, space="PSUM") as ypool,
            tc.tile_pool(name="rpool", bufs=2, space="PSUM") as rpool,
        ):
            cst = cpool.tile([128, 160], BF16)
            nc.sync.dma_start(out=cst[:], in_=cst_d[:])
            xct = cpool.tile([32, PC], BF16)
            nc.sync.dma_start(out=xct[:], in_=xc_d[:])

            ghA = cst[:, 0:64]
            RED24 = cst[:, 64:88]
            ghB = cst[0:32, 96:160]

            wha_t = []
            for i, (c0, c1) in enumerate(WCH):
                t = cpool.tile([128, c1 - c0], BF16, tag=f"wha{i}")
                nc.sync.dma_start(out=t[:], in_=wha_d[:, c0:c1])
                wha_t.append(t)
            whb_t = []
            for i, (c0, c1) in enumerate(BCH):
                t = cpool.tile([32, c1 - c0], BF16, tag=f"whb{i}")
                nc.gpsimd.dma_start(out=t[:], in_=whb_d[:, c0:c1])
                whb_t.append(t)

            # xd2: compact x table replicated 4x along partitions so the
            # multiply runs partition-tall.  Rows h*64 + k*16 + (d*4+b).
            # (stride-0 SBUF partition APs are rejected, so 8 plain DMAs
            # split across two issue engines)
            xd2 = cpool.tile([128, PC], BF16, tag="xd2")
            for h in range(2):
                for k in range(4):
                    r0 = h * 64 + 16 * k
                    eng = nc.gpsimd if h == 0 else nc.sync
                    eng.dma_start(out=xd2[r0:r0 + 16, :],
                                  in_=xct[16 * h:16 * h + 16, :])

            # PE p-state ramp: the clock starts ~0.8GHz and climbs only
            # under continuous execution; keep PE busy with cheap N=128
            # matmuls until the first W chunk lands (output never read).
            wsc = cpool.tile([128, 128], BF16, tag="wsc")
            nc.vector.memset(wsc[:], 0.0)
            ywarm = ypool.tile([64, 128], F32, tag="ywarm", bufs=1)
            for w in range(N_WARM):
                nc.tensor.matmul(ywarm[:], ghA, wsc[:],
                                 start=(w == 0), stop=(w == N_WARM - 1),
                                 skip_group_check=True)

            # DMA cannot read PSUM: RED-matmul results bounce through an
            # SBUF staging buffer via ACT copies.
            ro = cpool.tile([24, PC], F32, tag="ro")

            def emit_pair(p):
                y = ypool.tile([128, 512], F32, tag="y", bufs=3)
                for h in range(2):
                    g0 = 1024 * p + 512 * h
                    wa, sa = _locate(wha_t, WCH, g0, 512)
                    wb, sb = _locate(whb_t, BCH, g0, 512)
                    nc.tensor.matmul(y[64 * h:64 * h + 64, :], ghA,
                                     wa[:, sa], start=True, stop=False,
                                     skip_group_check=True)
                    nc.tensor.matmul(y[64 * h:64 * h + 64, :], ghB,
                                     wb[:, sb], start=False, stop=True,
                                     skip_group_check=True)
                s = spool.tile([128, 512], BF16, tag="s")
                nc.vector.tensor_mul(out=s[:], in0=y[:],
                                     in1=xd2[:, 512 * p:512 * p + 512])
                return s

            def emit_red(p, s):
                r = rpool.tile([24, 512], F32, tag="r", bufs=3)
                nc.tensor.matmul(r[:], RED24, s[:], start=True, stop=True,
                                 skip_group_check=True)
                nc.scalar.copy(out=ro[:, 512 * p:512 * p + 512], in_=r[:])
                if p == 2:
                    nc.sync.dma_start(out=outT[:, 0:1536], in_=ro[:, 0:1536])

            # software pipeline: RED(p) runs two pairs behind the A/B
            # matmuls so the PE never waits on the DVE multiply.
            s_tiles = {}
            for p in range(NPAIR):
                s_tiles[p] = emit_pair(p)
                if p >= 2:
                    emit_red(p - 2, s_tiles.pop(p - 2))

            # 128-vertex tail (single half), using pooled tile slices
            yt = ypool.tile([128, 512], F32, tag="y", bufs=3)
            wa, sa = _locate(wha_t, WCH, 6144, 128)
            wb, sb = _locate(whb_t, BCH, 6144, 128)
            nc.tensor.matmul(yt[0:64, 0:128], ghA, wa[:, sa],
                             start=True, stop=False, skip_group_check=True)
            nc.tensor.matmul(yt[0:64, 0:128], ghB, wb[:, sb],
                             start=False, stop=True, skip_group_check=True)
            st = spool.tile([128, 512], BF16, tag="s")
            nc.vector.tensor_mul(out=st[0:64, 0:128], in0=yt[0:64, 0:128],
                                 in1=xd2[0:64, 3072:3200])

            emit_red(4, s_tiles.pop(4))
            emit_red(5, s_tiles.pop(5))
            rt = rpool.tile([24, 512], F32, tag="r", bufs=3)
            nc.tensor.matmul(rt[:, 0:128], cst[0:64, 64:88], st[0:64, 0:128],
                             start=True, stop=True, skip_group_check=True)
            nc.scalar.copy(out=ro[:, 3072:3200], in_=rt[:, 0:128])
            nc.sync.dma_start(out=outT[:, 1536:3200], in_=ro[:, 1536:3200])
    nc.finalize()
    return nc


_NC_CACHE = None


def _get_nc():
    global _NC_CACHE
    if _NC_CACHE is None:
        _NC_CACHE = _build_bass()
    return _NC_CACHE


def _host_prep(X, V_nodes, rot6d_nodes, W_nodes, idx_nn_to_nodes):
    """Small per-node math (B*C=640 rows) + shard/layout of the big tensors."""
    X = np.asarray(X, np.float32)
    Vn = np.asarray(V_nodes, np.float32)
    d6 = np.asarray(rot6d_nodes, np.float32)
    W = np.asarray(W_nodes, np.float32)
    idx = np.asarray(idx_nn_to_nodes).astype(np.int64)

    a1, a2 = d6[..., :3], d6[..., 3:]
    eps = np.float32(1e-8)
    n1 = np.sqrt(np.sum(a1 * a1, -1, keepdims=True, dtype=np.float32))
    b1 = a1 / np.maximum(n1, eps)
    dot = np.sum(b1 * a2, -1, keepdims=True, dtype=np.float32)
    a2p = a2 - dot * b1
    n2 = np.sqrt(np.sum(a2p * a2p, -1, keepdims=True, dtype=np.float32))
    b2 = a2p / np.maximum(n2, eps)
    b3 = np.cross(b1, b2)
    R = np.stack([b1, b2, b3], axis=-2).astype(np.float32)  # (B,C,3,3) [b,c,k,d]

    center = X[:, idx, :]                                   # (B,C,3)
    t = (center + Vn - np.einsum('bcd,bckd->bck', center, R)).astype(np.float32)

    # G columns at j = k*16 + d*4 + b (d==3 = translation); cols 48:64 zero
    Gv = np.zeros((C, 4, 4, 4), np.float32)
    Gv[:, 0:3, 0:3, :] = np.transpose(R, (1, 2, 3, 0))
    Gv[:, 0:3, 3, :] = np.transpose(t, (1, 2, 0))
    G = Gv.reshape(C, 64)

    RED = np.zeros((2, 4, 4, 4, 24), np.float32)
    for h in range(2):
        for k in range(3):
            for b in range(B):
                RED[h, k, :, b, h * 12 + k * 4 + b] = 1.0
    RED = RED.reshape(128, 24)

    cst = np.zeros((128, 160), NPBF16)
    cst[:, 0:64] = G[0:128].astype(NPBF16)
    cst[:, 64:88] = RED.astype(NPBF16)
    cst[0:32, 96:160] = G[128:160].astype(NPBF16)

    Wb = W.astype(NPBF16)
    in_maps = []
    for i in range(N_CORES):
        vsl = slice(i * VS, (i + 1) * VS)
        wt = np.zeros((160, VSP), NPBF16)
        wt[:, :VS] = Wb[vsl].T
        wha = np.ascontiguousarray(wt[0:128])
        whb = np.ascontiguousarray(wt[128:160])

        Xs = np.zeros((B, VSP, 3), np.float32)
        Xs[:, :VS] = X[:, vsl, :]
        xc = np.zeros((2, 4, 4, PC), np.float32)        # [h, d, b, col]
        main = Xs[:, :6144].reshape(B, NPAIR, 2, 512, 3)
        xc[:, 0:3, :, 0:3072] = np.transpose(
            main, (2, 4, 0, 1, 3)).reshape(2, 3, B, 3072)
        xc[:, 3, :, 0:3072] = 1.0
        xc[0, 0:3, :, 3072:3200] = np.transpose(Xs[:, 6144:6272], (2, 0, 1))
        xc[0, 3, :, 3072:3200] = 1.0
        xc = np.ascontiguousarray(xc.reshape(32, PC).astype(NPBF16))

        in_maps.append({"cst": cst, "wha": wha, "whb": whb, "xc": xc})
    return in_maps


def _gather(results):
    out = np.empty((B, V, 3), np.float32)
    for i, res in enumerate(results):
        o = res["outT"].reshape(2, 3, 4, PC)            # [h, k, b, col]
        om = o[:, :, :, 0:3072].reshape(2, 3, 4, NPAIR, 512)
        block = np.empty((B, VSP, 3), np.float32)
        block[:, :6144] = np.transpose(om, (2, 3, 0, 4, 1)).reshape(B, 6144, 3)
        block[:, 6144:6272] = np.transpose(o[0, :, :, 3072:3200], (1, 2, 0))
        out[:, i * VS:(i + 1) * VS] = block[:, :VS]
    return out


def kernel(X, V_nodes, rot6d_nodes, W_nodes, idx_nn_to_nodes, **run_kwargs):
    in_maps = _host_prep(X, V_nodes, rot6d_nodes, W_nodes, idx_nn_to_nodes)
    res = run_bass_kernel_spmd(_get_nc(), in_maps,
                               core_ids=list(range(N_CORES)), **run_kwargs)
    out = _gather(res.results)
    kernel.last_run = res
    return out


# revision 15
# speedup vs baseline: 1.3363x; 1.0660x over previous
"""Trainium2 Bass kernel for the DeformationGraph problem.

Math: per batch b and vertex v,
    out[b,v,k] = sum_c W[v,c] * ( sum_d (X[b,v,d]-center[b,c,d]) * R[b,c,k,d]
                                  + center[b,c,k] + V_nodes[b,c,k] )
factors into a vertex-independent per-node affine map:
    t[b,c,k]   = center[b,c,k] + V_nodes[b,c,k] - sum_d center[b,c,d]*R[b,c,k,d]
    out[b,v,k] = sum_d X[b,v,d] * (W @ R[..,k,d])[v]  +  (W @ t[..,k])[v]
i.e. one (V,C)@(C,48) matmul Y = W @ G, then a per-vertex contraction of Y
with [X,1].  W/X/out are sharded over the vertex dim across 8 cores.

Precision: rel-err budget is 2e-2; a single bf16 term (W, G, x, and the
product tensor all bf16, fp32 accumulation) measures ~3e-3 end-to-end,
so no multi-term splits are used.

Per-core pipeline (vertex shard padded to 6272 = 6*1024 + 128):
  - PE: per 1024-vertex pair, y[0:64] and y[64:128] in one PSUM tile get
    (K=128 "A" + K=32 "B") accumulated matmuls (G-column layout
    j = k*16 + d*4 + b, d==3 = translation, cols 48:64 zero).
  - DVE: one [128,512] tensor_mul  s = y * xd2  (bf16 out to SBUF).
    xd2 is the compact per-vertex [X,1] table replicated 4x along
    partitions by SBUF->SBUF DMAs so the multiply is partition-tall
    (engine op cost scales with free-dim columns only).
  - PE again: the 4-way d-reduction runs as a 0/1 "reduction matmul"
    r[24, n] = RED^T @ s  (rows h*12 + k*4 + b), output PSUM, DMA'd
    straight to DRAM.  This keeps DVE at one op per 1024 vertices and
    leaves ACT/Pool free for DMA issue.
HBM traffic/core: ~1.6MB W_A + 0.4MB W_B + 0.2MB x + 0.3MB out = 2.5MB.
"""

import numpy as np
import ml_dtypes

import concourse.mybir as mybir
import concourse.tile as tile
from concourse import bacc
from concourse.bass_utils import run_bass_kernel_spmd

B, V, C = 4, 50000, 160
N_CORES = 8
VS = V // N_CORES            # 6250 vertices per core
VSP = 6272                   # padded shard: 6 pairs of 1024 + 128 tail
NPAIR = 6
PC = 3200                    # pair-col space: 6*512 + 128
F32 = mybir.dt.float32
BF16 = mybir.dt.bfloat16
NPBF16 = ml_dtypes.bfloat16

WCH = [(0, 1024), (1024, 3072), (3072, VSP)]   # wha DMA chunks
BCH = [(0, 1024), (1024, VSP)]                 # whb DMA chunks
N_WARM = 40                                    # N=128 ramp matmuls


def _locate(tiles, chunks, g0, width):
    for t, (c0, c1) in zip(tiles, chunks):
        if c0 <= g0 and g0 + width <= c1:
            return t, slice(g0 - c0, g0 - c0 + width)
    raise AssertionError(f"col range {g0}+{width} crosses chunk boundary")


def _build_bass():
    nc = bacc.Bacc()

    cst_d = nc.dram_tensor("cst", [128, 160], BF16, kind="ExternalInput")
    wha_d = nc.dram_tensor("wha", [128, VSP], BF16, kind="ExternalInput")
    whb_d = nc.dram_tensor("whb", [32, VSP], BF16, kind="ExternalInput")
    xc_d = nc.dram_tensor("xc", [64, PC], BF16, kind="ExternalInput")
    outT = nc.dram_tensor("outT", [24, PC], F32, kind="ExternalOutput")

    with tile.TileContext(nc) as tc:
        with (
            tc.tile_pool(name="cpool", bufs=1) as cpool,
            tc.tile_pool(name="spool", bufs=3) as spool,
            tc.tile_pool(name="ypool", bufs=2, space="PSUM") as ypool,
            tc.tile_pool(name="rpool", bufs=2, space="PSUM") as rpool,
        ):
            cst = cpool.tile([128, 160], BF16)
            nc.sync.dma_start(out=cst[:], in_=cst_d[:])

            ghA = cst[:, 0:64]
            RED24 = cst[:, 64:88]
            ghB = cst[0:32, 96:160]

            # xd2: per-vertex [X,1] table, 4 copies along partitions so the
            # multiply runs partition-tall.  Rows h*64 + k*16 + (d*4+b).
            # Host ships 2 copies (xc [64, PC]); the second doubling is one
            # SBUF->SBUF DMA per half.  Issued before the W streams so the
            # first multiply is never gated on the bulk W traffic.
            xd2 = cpool.tile([128, PC], BF16, tag="xd2")
            for h in range(2):
                nc.sync.dma_start(out=xd2[h * 64:h * 64 + 32, :],
                                  in_=xc_d[32 * h:32 * h + 32, :])
            for h in range(2):
                nc.gpsimd.dma_start(out=xd2[h * 64 + 32:h * 64 + 64, :],
                                    in_=xd2[h * 64:h * 64 + 32, :])

            wha_t = []
            for i, (c0, c1) in enumerate(WCH):
                t = cpool.tile([128, c1 - c0], BF16, tag=f"wha{i}")
                nc.sync.dma_start(out=t[:], in_=wha_d[:, c0:c1])
                wha_t.append(t)
            whb_t = []
            for i, (c0, c1) in enumerate(BCH):
                t = cpool.tile([32, c1 - c0], BF16, tag=f"whb{i}")
                nc.gpsimd.dma_start(out=t[:], in_=whb_d[:, c0:c1])
                whb_t.append(t)

            # PE p-state ramp: the clock starts ~0.8GHz and climbs only
            # under continuous execution; keep PE busy with cheap N=128
            # matmuls until the first W chunk lands (output never read).
            wsc = cpool.tile([128, 128], BF16, tag="wsc")
            nc.vector.memset(wsc[:], 0.0)
            ywarm = ypool.tile([64, 128], F32, tag="ywarm", bufs=1)
            for w in range(N_WARM):
                nc.tensor.matmul(ywarm[:], ghA, wsc[:],
                                 start=(w == 0), stop=(w == N_WARM - 1),
                                 skip_group_check=True)

            # DMA cannot read PSUM: RED-matmul results bounce through an
            # SBUF staging buffer via ACT copies.
            ro = cpool.tile([24, PC], F32, tag="ro")

            def emit_pair(p):
                y = ypool.tile([128, 512], F32, tag="y", bufs=3)
                for h in range(2):
                    g0 = 1024 * p + 512 * h
                    wa, sa = _locate(wha_t, WCH, g0, 512)
                    wb, sb = _locate(whb_t, BCH, g0, 512)
                    nc.tensor.matmul(y[64 * h:64 * h + 64, :], ghA,
                                     wa[:, sa], start=True, stop=False,
                                     skip_group_check=True)
                    nc.tensor.matmul(y[64 * h:64 * h + 64, :], ghB,
                                     wb[:, sb], start=False, stop=True,
                                     skip_group_check=True)
                s = spool.tile([128, 512], BF16, tag="s")
                nc.vector.tensor_mul(out=s[:], in0=y[:],
                                     in1=xd2[:, 512 * p:512 * p + 512])
                return s

            def emit_red(p, s):
                r = rpool.tile([24, 512], F32, tag="r", bufs=3)
                nc.tensor.matmul(r[:], RED24, s[:], start=True, stop=True,
                                 skip_group_check=True)
                nc.scalar.copy(out=ro[:, 512 * p:512 * p + 512], in_=r[:])
                if p % 2 == 1:
                    c0 = 1024 * (p // 2)
                    nc.sync.dma_start(out=outT[:, c0:c0 + 1024],
                                      in_=ro[:, c0:c0 + 1024])

            # software pipeline: RED(p) runs two pairs behind the A/B
            # matmuls so the PE never waits on the DVE multiply.
            s_tiles = {}
            for p in range(NPAIR):
                s_tiles[p] = emit_pair(p)
                if p >= 2:
                    emit_red(p - 2, s_tiles.pop(p - 2))

            # 128-vertex tail (single half), using pooled tile slices
            yt = ypool.tile([128, 512], F32, tag="y", bufs=3)
            wa, sa = _locate(wha_t, WCH, 6144, 128)
            wb, sb = _locate(whb_t, BCH, 6144, 128)
            nc.tensor.matmul(yt[0:64, 0:128], ghA, wa[:, sa],
                             start=True, stop=False, skip_group_check=True)
            nc.tensor.matmul(yt[0:64, 0:128], ghB, wb[:, sb],
                             start=False, stop=True, skip_group_check=True)
            st = spool.tile([128, 512], BF16, tag="s")
            nc.vector.tensor_mul(out=st[0:64, 0:128], in0=yt[0:64, 0:128],
                                 in1=xd2[0:64, 3072:3200])

            emit_red(4, s_tiles.pop(4))
            emit_red(5, s_tiles.pop(5))
            rt = rpool.tile([24, 512], F32, tag="r", bufs=3)
            nc.tensor.matmul(rt[:, 0:128], cst[0:64, 64:88], st[0:64, 0:128],
                             start=True, stop=True, skip_group_check=True)
            nc.scalar.copy(out=ro[:, 3072:3200], in_=rt[:, 0:128])
            nc.sync.dma_start(out=outT[:, 3072:3200], in_=ro[:, 3072:3200])
    nc.finalize()
    return nc


_NC_CACHE = None


def _get_nc():
    global _NC_CACHE
    if _NC_CACHE is None:
        _NC_CACHE = _build_bass()
    return _NC_CACHE


def _host_prep(X, V_nodes, rot6d_nodes, W_nodes, idx_nn_to_nodes):
    """Small per-node math (B*C=640 rows) + shard/layout of the big tensors."""
    X = np.asarray(X, np.float32)
    Vn = np.asarray(V_nodes, np.float32)
    d6 = np.asarray(rot6d_nodes, np.float32)
    W = np.asarray(W_nodes, np.float32)
    idx = np.asarray(idx_nn_to_nodes).astype(np.int64)

    a1, a2 = d6[..., :3], d6[..., 3:]
    eps = np.float32(1e-8)
    n1 = np.sqrt(np.sum(a1 * a1, -1, keepdims=True, dtype=np.float32))
    b1 = a1 / np.maximum(n1, eps)
    dot = np.sum(b1 * a2, -1, keepdims=True, dtype=np.float32)
    a2p = a2 - dot * b1
    n2 = np.sqrt(np.sum(a2p * a2p, -1, keepdims=True, dtype=np.float32))
    b2 = a2p / np.maximum(n2, eps)
    b3 = np.cross(b1, b2)
    R = np.stack([b1, b2, b3], axis=-2).astype(np.float32)  # (B,C,3,3) [b,c,k,d]

    center = X[:, idx, :]                                   # (B,C,3)
    t = (center + Vn - np.einsum('bcd,bckd->bck', center, R)).astype(np.float32)

    # G columns at j = k*16 + d*4 + b (d==3 = translation); cols 48:64 zero
    Gv = np.zeros((C, 4, 4, 4), np.float32)
    Gv[:, 0:3, 0:3, :] = np.transpose(R, (1, 2, 3, 0))
    Gv[:, 0:3, 3, :] = np.transpose(t, (1, 2, 0))
    G = Gv.reshape(C, 64)

    RED = np.zeros((2, 4, 4, 4, 24), np.float32)
    for h in range(2):
        for k in range(3):
            for b in range(B):
                RED[h, k, :, b, h * 12 + k * 4 + b] = 1.0
    RED = RED.reshape(128, 24)

    cst = np.zeros((128, 160), NPBF16)
    cst[:, 0:64] = G[0:128].astype(NPBF16)
    cst[:, 64:88] = RED.astype(NPBF16)
    cst[0:32, 96:160] = G[128:160].astype(NPBF16)

    Wb = W.astype(NPBF16)
    in_maps = []
    for i in range(N_CORES):
        vsl = slice(i * VS, (i + 1) * VS)
        wt = np.zeros((160, VSP), NPBF16)
        wt[:, :VS] = Wb[vsl].T
        wha = np.ascontiguousarray(wt[0:128])
        whb = np.ascontiguousarray(wt[128:160])

        Xs = np.zeros((B, VSP, 3), np.float32)
        Xs[:, :VS] = X[:, vsl, :]
        xc = np.zeros((2, 4, 4, PC), np.float32)        # [h, d, b, col]
        main = Xs[:, :6144].reshape(B, NPAIR, 2, 512, 3)
        xc[:, 0:3, :, 0:3072] = np.transpose(
            main, (2, 4, 0, 1, 3)).reshape(2, 3, B, 3072)
        xc[:, 3, :, 0:3072] = 1.0
        xc[0, 0:3, :, 3072:3200] = np.transpose(Xs[:, 6144:6272], (2, 0, 1))
        xc[0, 3, :, 3072:3200] = 1.0
        xc = xc.reshape(2, 16, PC)
        # ship two copies per half ([64, PC]); kernel doubles once more
        xc = np.ascontiguousarray(
            np.concatenate([xc[0], xc[0], xc[1], xc[1]], 0).astype(NPBF16))

        in_maps.append({"cst": cst, "wha": wha, "whb": whb, "xc": xc})
    return in_maps


def _gather(results):
    out = np.empty((B, V, 3), np.float32)
    for i, res in enumerate(results):
        o = res["outT"].reshape(2, 3, 4, PC)            # [h, k, b, col]
        om = o[:, :, :, 0:3072].reshape(2, 3, 4, NPAIR, 512)
        block = np.empty((B, VSP, 3), np.float32)
        block[:, :6144] = np.transpose(om, (2, 3, 0, 4, 1)).reshape(B, 6144, 3)
        block[:, 6144:6272] = np.transpose(o[0, :, :, 3072:3200], (1, 2, 0))
        out[:, i * VS:(i + 1) * VS] = block[:, :VS]
    return out


def kernel(X, V_nodes, rot6d_nodes, W_nodes, idx_nn_to_nodes, **run_kwargs):
    in_maps = _host_prep(X, V_nodes, rot6d_nodes, W_nodes, idx_nn_to_nodes)
    res = run_bass_kernel_spmd(_get_nc(), in_maps,
                               core_ids=list(range(N_CORES)), **run_kwargs)
    out = _gather(res.results)
    kernel.last_run = res
    return out


# revision 22
# speedup vs baseline: 1.3955x; 1.0443x over previous
"""Trainium2 Bass kernel for the DeformationGraph problem.

Math: per batch b and vertex v,
    out[b,v,k] = sum_c W[v,c] * ( sum_d (X[b,v,d]-center[b,c,d]) * R[b,c,k,d]
                                  + center[b,c,k] + V_nodes[b,c,k] )
factors into a vertex-independent per-node affine map:
    t[b,c,k]   = center[b,c,k] + V_nodes[b,c,k] - sum_d center[b,c,d]*R[b,c,k,d]
    out[b,v,k] = sum_d X[b,v,d] * (W @ R[..,k,d])[v]  +  (W @ t[..,k])[v]
i.e. one (V,C)@(C,48) matmul Y = W @ G, then a per-vertex contraction of Y
with [X,1].  W/X/out are sharded over the vertex dim across 8 cores.

Precision: rel-err budget is 2e-2; a single bf16 term (W, G, x, and the
product tensor all bf16, fp32 accumulation) measures ~3e-3 end-to-end,
so no multi-term splits are used.

Per-core pipeline (vertex shard padded to 6272 = 6*1024 + 128):
  - PE: per 1024-vertex pair, y[0:64] and y[64:128] in one PSUM tile get
    (K=128 "A" + K=32 "B") accumulated matmuls (G-column layout
    j = k*16 + d*4 + b, d==3 = translation, cols 48:64 zero).
  - DVE: one [128,512] tensor_mul  s = y * xd2  (bf16 out to SBUF).
    xd2 is the compact per-vertex [X,1] table replicated 4x along
    partitions by SBUF->SBUF DMAs so the multiply is partition-tall
    (engine op cost scales with free-dim columns only).
  - PE again: the 4-way d-reduction runs as a 0/1 "reduction matmul"
    r[24, n] = RED^T @ s  (rows h*12 + k*4 + b), output PSUM, DMA'd
    straight to DRAM.  This keeps DVE at one op per 1024 vertices and
    leaves ACT/Pool free for DMA issue.
HBM traffic/core: ~1.6MB W_A + 0.4MB W_B + 0.2MB x + 0.3MB out = 2.5MB.
"""

import numpy as np
import ml_dtypes

import concourse.mybir as mybir
import concourse.tile as tile
from concourse import bacc
from concourse.bass_utils import run_bass_kernel_spmd

B, V, C = 4, 50000, 160
N_CORES = 8
VS = V // N_CORES            # 6250 vertices per core
VSP = 6272                   # padded shard: 6 pairs of 1024 + 128 tail
NPAIR = 6
PC = 3200                    # pair-col space: 6*512 + 128
F32 = mybir.dt.float32
BF16 = mybir.dt.bfloat16
NPBF16 = ml_dtypes.bfloat16

WCH = [(0, 2048), (2048, 4096), (4096, VSP)]   # wha DMA chunks (vertex cols)
BCH = [(0, 1024), (1024, PC)]                  # whb2 DMA chunks (pair cols)
N_WARM = 40                                    # N=128 ramp matmuls


def _locate(tiles, chunks, g0, width):
    for t, (c0, c1) in zip(tiles, chunks):
        if c0 <= g0 and g0 + width <= c1:
            return t, slice(g0 - c0, g0 - c0 + width)
    raise AssertionError(f"col range {g0}+{width} crosses chunk boundary")


def _build_bass():
    nc = bacc.Bacc()

    cst_d = nc.dram_tensor("cst", [128, 224], BF16, kind="ExternalInput")
    wha_d = nc.dram_tensor("wha", [128, VSP], BF16, kind="ExternalInput")
    whb_d = nc.dram_tensor("whb", [64, PC], BF16, kind="ExternalInput")
    xc_d = nc.dram_tensor("xc", [64, PC], BF16, kind="ExternalInput")
    outT = nc.dram_tensor("outT", [24, PC], F32, kind="ExternalOutput")

    with tile.TileContext(nc) as tc:
        with (
            tc.tile_pool(name="cpool", bufs=1) as cpool,
            tc.tile_pool(name="spool", bufs=3) as spool,
            tc.tile_pool(name="ypool", bufs=2, space="PSUM") as ypool,
            tc.tile_pool(name="rpool", bufs=2, space="PSUM") as rpool,
        ):
            cst = cpool.tile([128, 224], BF16)
            nc.sync.dma_start(out=cst[:], in_=cst_d[:])

            ghA = cst[:, 0:64]
            RED24 = cst[:, 64:88]
            ghB2 = cst[0:64, 96:224]   # block-diag [[G_B,0],[0,G_B]]

            # xd2: per-vertex [X,1] table, 4 copies along partitions so the
            # multiply runs partition-tall.  Rows h*64 + k*16 + (d*4+b).
            # Host ships 2 copies (xc [64, PC]); the second doubling is one
            # SBUF->SBUF DMA per half.  All on the sync HWDGE queue, ahead
            # of the bulk W streams, so the first multiply is never gated.
            xd2 = cpool.tile([128, PC], BF16, tag="xd2")
            for h in range(2):
                nc.sync.dma_start(out=xd2[h * 64:h * 64 + 32, :],
                                  in_=xc_d[32 * h:32 * h + 32, :])
            for h in range(2):
                nc.sync.dma_start(out=xd2[h * 64 + 32:h * 64 + 64, :],
                                  in_=xd2[h * 64:h * 64 + 32, :])

            # W streams spread across three issue queues: whb2 chunk 0 on
            # sync (small, needed first), wha on scalar's HWDGE queue,
            # bulky whb2 chunk 1 on gpsimd's SWDGE (latency-tolerant).
            whb_t = []
            for i, (c0, c1) in enumerate(BCH):
                t = cpool.tile([64, c1 - c0], BF16, tag=f"whb{i}")
                (nc.sync if i == 0 else nc.gpsimd).dma_start(
                    out=t[:], in_=whb_d[:, c0:c1])
                whb_t.append(t)
            wha_t = []
            for i, (c0, c1) in enumerate(WCH):
                t = cpool.tile([128, c1 - c0], BF16, tag=f"wha{i}")
                nc.scalar.dma_start(out=t[:], in_=wha_d[:, c0:c1])
                wha_t.append(t)

            # PE p-state ramp: the clock starts ~0.8GHz and climbs only
            # under continuous execution; keep PE busy with cheap N=128
            # matmuls until the first W chunk lands (output never read).
            wsc = cpool.tile([128, 128], BF16, tag="wsc")
            nc.vector.memset(wsc[:], 0.0)
            ywarm = ypool.tile([64, 128], F32, tag="ywarm", bufs=1)
            for w in range(N_WARM):
                nc.tensor.matmul(ywarm[:], ghA, wsc[:],
                                 start=(w == 0), stop=(w == N_WARM - 1),
                                 skip_group_check=True)

            # DMA cannot read PSUM: RED-matmul results bounce through an
            # SBUF staging buffer via ACT copies.
            ro = cpool.tile([24, PC], F32, tag="ro")

            def emit_pair(p):
                y = ypool.tile([128, 512], F32, tag="y", bufs=3)
                for h in range(2):
                    g0 = 1024 * p + 512 * h
                    wa, sa = _locate(wha_t, WCH, g0, 512)
                    nc.tensor.matmul(y[64 * h:64 * h + 64, :], ghA,
                                     wa[:, sa], start=True, stop=False,
                                     skip_group_check=True)
                wb, sb = _locate(whb_t, BCH, 512 * p, 512)
                nc.tensor.matmul(y[:], ghB2, wb[:, sb],
                                 start=False, stop=True,
                                 skip_group_check=True)
                s = spool.tile([128, 512], BF16, tag="s")
                nc.vector.tensor_mul(out=s[:], in0=y[:],
                                     in1=xd2[:, 512 * p:512 * p + 512])
                return s

            def emit_red(p, s):
                r = rpool.tile([24, 512], F32, tag="r", bufs=3)
                nc.tensor.matmul(r[:], RED24, s[:], start=True, stop=True,
                                 skip_group_check=True)
                nc.scalar.copy(out=ro[:, 512 * p:512 * p + 512], in_=r[:])
                if p % 2 == 1:
                    c0 = 1024 * (p // 2)
                    nc.sync.dma_start(out=outT[:, c0:c0 + 1024],
                                      in_=ro[:, c0:c0 + 1024])

            # software pipeline: RED(p) runs two pairs behind the A/B
            # matmuls so the PE never waits on the DVE multiply.
            s_tiles = {}
            for p in range(NPAIR):
                s_tiles[p] = emit_pair(p)
                if p >= 2:
                    emit_red(p - 2, s_tiles.pop(p - 2))

            # 128-vertex tail (single half), using pooled tile slices
            yt = ypool.tile([128, 512], F32, tag="y", bufs=3)
            wa, sa = _locate(wha_t, WCH, 6144, 128)
            wb, sb = _locate(whb_t, BCH, 3072, 128)
            nc.tensor.matmul(yt[0:64, 0:128], ghA, wa[:, sa],
                             start=True, stop=False, skip_group_check=True)
            nc.tensor.matmul(yt[0:64, 0:128], ghB2[:, 0:64], wb[:, sb],
                             start=False, stop=True, skip_group_check=True)
            st = spool.tile([128, 512], BF16, tag="s")
            nc.vector.tensor_mul(out=st[0:64, 0:128], in0=yt[0:64, 0:128],
                                 in1=xd2[0:64, 3072:3200])

            emit_red(4, s_tiles.pop(4))
            emit_red(5, s_tiles.pop(5))
            rt = rpool.tile([24, 512], F32, tag="r", bufs=3)
            nc.tensor.matmul(rt[:, 0:128], cst[0:64, 64:88], st[0:64, 0:128],
                             start=True, stop=True, skip_group_check=True)
            nc.scalar.copy(out=ro[:, 3072:3200], in_=rt[:, 0:128])
            nc.sync.dma_start(out=outT[:, 3072:3200], in_=ro[:, 3072:3200])
    nc.finalize()
    return nc


_NC_CACHE = None


def _get_nc():
    global _NC_CACHE
    if _NC_CACHE is None:
        _NC_CACHE = _build_bass()
    return _NC_CACHE


def _host_prep(X, V_nodes, rot6d_nodes, W_nodes, idx_nn_to_nodes):
    """Small per-node math (B*C=640 rows) + shard/layout of the big tensors."""
    X = np.asarray(X, np.float32)
    Vn = np.asarray(V_nodes, np.float32)
    d6 = np.asarray(rot6d_nodes, np.float32)
    W = np.asarray(W_nodes, np.float32)
    idx = np.asarray(idx_nn_to_nodes).astype(np.int64)

    a1, a2 = d6[..., :3], d6[..., 3:]
    eps = np.float32(1e-8)
    n1 = np.sqrt(np.sum(a1 * a1, -1, keepdims=True, dtype=np.float32))
    b1 = a1 / np.maximum(n1, eps)
    dot = np.sum(b1 * a2, -1, keepdims=True, dtype=np.float32)
    a2p = a2 - dot * b1
    n2 = np.sqrt(np.sum(a2p * a2p, -1, keepdims=True, dtype=np.float32))
    b2 = a2p / np.maximum(n2, eps)
    b3 = np.cross(b1, b2)
    R = np.stack([b1, b2, b3], axis=-2).astype(np.float32)  # (B,C,3,3) [b,c,k,d]

    center = X[:, idx, :]                                   # (B,C,3)
    t = (center + Vn - np.einsum('bcd,bckd->bck', center, R)).astype(np.float32)

    # G columns at j = k*16 + d*4 + b (d==3 = translation); cols 48:64 zero
    Gv = np.zeros((C, 4, 4, 4), np.float32)
    Gv[:, 0:3, 0:3, :] = np.transpose(R, (1, 2, 3, 0))
    Gv[:, 0:3, 3, :] = np.transpose(t, (1, 2, 0))
    G = Gv.reshape(C, 64)

    RED = np.zeros((2, 4, 4, 4, 24), np.float32)
    for h in range(2):
        for k in range(3):
            for b in range(B):
                RED[h, k, :, b, h * 12 + k * 4 + b] = 1.0
    RED = RED.reshape(128, 24)

    cst = np.zeros((128, 224), NPBF16)
    cst[:, 0:64] = G[0:128].astype(NPBF16)
    cst[:, 64:88] = RED.astype(NPBF16)
    gB = G[128:160].astype(NPBF16)             # [32, 64]
    cst[0:32, 96:160] = gB                     # even-half block
    cst[32:64, 160:224] = gB                   # odd-half block

    Wb = W.astype(NPBF16)
    in_maps = []
    for i in range(N_CORES):
        vsl = slice(i * VS, (i + 1) * VS)
        wt = np.zeros((160, VSP), NPBF16)
        wt[:, :VS] = Wb[vsl].T
        wha = np.ascontiguousarray(wt[0:128])
        # B part in vertex-pair-column layout: whb[(h*32+c), 512p+j] =
        # W_B[c, 1024p + 512h + j]; tail (cols 3072:3200) even-half only
        bp = wt[128:160]                       # [32, VSP]
        whb = np.zeros((64, PC), NPBF16)
        whb[:, 0:3072] = bp[:, :6144].reshape(32, NPAIR, 2, 512).transpose(
            2, 0, 1, 3).reshape(64, 3072)
        whb[0:32, 3072:3200] = bp[:, 6144:6272]

        Xs = np.zeros((B, VSP, 3), np.float32)
        Xs[:, :VS] = X[:, vsl, :]
        xc = np.zeros((2, 4, 4, PC), np.float32)        # [h, d, b, col]
        main = Xs[:, :6144].reshape(B, NPAIR, 2, 512, 3)
        xc[:, 0:3, :, 0:3072] = np.transpose(
            main, (2, 4, 0, 1, 3)).reshape(2, 3, B, 3072)
        xc[:, 3, :, 0:3072] = 1.0
        xc[0, 0:3, :, 3072:3200] = np.transpose(Xs[:, 6144:6272], (2, 0, 1))
        xc[0, 3, :, 3072:3200] = 1.0
        xc = xc.reshape(2, 16, PC)
        # ship two copies per half ([64, PC]); kernel doubles once more
        xc = np.ascontiguousarray(
            np.concatenate([xc[0], xc[0], xc[1], xc[1]], 0).astype(NPBF16))

        in_maps.append({"cst": cst, "wha": wha, "whb": whb, "xc": xc})
    return in_maps


def _gather(results):
    out = np.empty((B, V, 3), np.float32)
    for i, res in enumerate(results):
        o = res["outT"].reshape(2, 3, 4, PC)            # [h, k, b, col]
        om = o[:, :, :, 0:3072].reshape(2, 3, 4, NPAIR, 512)
        block = np.empty((B, VSP, 3), np.float32)
        block[:, :6144] = np.transpose(om, (2, 3, 0, 4, 1)).reshape(B, 6144, 3)
        block[:, 6144:6272] = np.transpose(o[0, :, :, 3072:3200], (1, 2, 0))
        out[:, i * VS:(i + 1) * VS] = block[:, :VS]
    return out


def kernel(X, V_nodes, rot6d_nodes, W_nodes, idx_nn_to_nodes, **run_kwargs):
    in_maps = _host_prep(X, V_nodes, rot6d_nodes, W_nodes, idx_nn_to_nodes)
    res = run_bass_kernel_spmd(_get_nc(), in_maps,
                               core_ids=list(range(N_CORES)), **run_kwargs)
    out = _gather(res.results)
    kernel.last_run = res
    return out
